# revision 38
# baseline (speedup 1.0000x reference)
"""Chunked-causal attention (MemoryEfficientAttention) for Trainium2.

Full inputs q,k,v: [2, 16, 2048, 64] fp32. Causal attention per (batch, head).
Sharding: 32 (batch*head) slices split 4-per-core across 8 NeuronCores.

Per-core design (v4). Three engine budgets are balanced against each other:
PE ~45us of matmul columns + weight loads, and the ~8.9M-element exp over
causal scores split between ScalarE (exact ACTIVATE Exp) and DVE
(Schraudolph int16-bitcast fp16 approx) so neither engine exceeds ~50us.

  - Host prep (free off-chip): q,k cast to bf16, d-major, and DUPLICATED on
    both partition halves (qdup[128, 2048]; kd1 = key blocks 0-7 on both
    halves, kd2 = blocks 8-15). k is PRE-SCALED by SCHR_A so the DVE fast
    exp is a single-op tensor_scalar. v ships fp16. Per-tile DMAs are
    atomic, so both PE row groups become ready simultaneously (skewed
    readiness makes the Tile scheduler bunch one row group's matmuls,
    serializing the PE).
  - QK^T: K=64 matmuls on the two 64-row PE groups (tile_position (0,0) /
    (64,0)); the groups execute concurrently when consecutive instructions
    alternate row group, so QK tasks are assigned by a running
    column-balance and emitted strictly alternating (cross-slab leftover
    deques). Scores land in 512-col slots of two 3-bank psum slabs
    (double buffered).
  - exp: per slab, greedy-balanced between ScalarE ACTIVATE(Exp) -> fp16
    eT and DVE tensor_scalar add(SCHR_B) bitcast fp16 (Schraudolph fast
    exp, ~1.5% element error). Chunk-0 slabs (short softmax windows) are
    forced to the exact ScalarE path; measured whole-problem rel err ~6e-3
    vs the 2e-2 gate. Each slab's exp is DEFERRED one slab so every QK
    writer is emitted before the psum read (else: fatal TensorE-write /
    ScalarE-read bank collision). Diagonal 128x128 blocks masked by a
    triangular fp16 multiply on the otherwise-idle GpSimd engine.
  - AV: ONE full-array matmul per task (K=128 keys, M=65 with row 64 = the
    ones-column softmax denominator) into a double-buffered [65, 512]
    accumulator -- half the weight loads of a row-group-split AV, no
    combine add, and no write-after-read stall at chunk boundaries.
    Epilogue: one ScalarE copy psum->SBUF, DMA out fp32; divide +
    transpose on host.
  - Software pipeline: AV for slab s-LAG is emitted next to QK for slab s;
    heads prefetch one ahead; chunks run in order (1,2,3,0) so the tiny
    chunk 0 drains the pipeline. Warmup matmuls bridge the head-0 load
    window (PE clock gate needs ~3.4us of sustained activity); a dummy
    ACTIVATE preloads the exp table set.

Softmax computed without max-subtraction: scores/8 stay well inside
fp32/fp16 exp range for this problem family (|q.k|/8 <~ 6 -> exp <= ~400).
"""

import hashlib
import os

import numpy as np

B, H, S, D = 2, 16, 2048, 64
N_CORES = 8
HPC = (B * H) // N_CORES  # heads per core
NB = S // 128             # 16 key/query blocks per head
LAG = 5                   # AV trails QK by this many slabs

_NC = None

# fast-exp constants: int16(round(A*s + B)) bitcast fp16 ~= exp(s/8).
# A is folded into k on the host; the kernel sees pre-scaled scores.
SCHR_A = 0.125 * 1.4426950408889634 * 1024.0
SCHR_B = 15360.0 - 44.0


def _install_neff_cache():
    """Content-addressed NEFF cache so repeat runs skip the walrus compile."""
    import concourse.bass2jax as bass2jax

    real_compile = bass2jax.compile_bir_kernel
    if getattr(bass2jax, "_neff_cache_installed", False):
        return
    cache_dir = os.path.expanduser("~/.cache/bass_neff")
    os.makedirs(cache_dir, exist_ok=True)

    def cached_compile(bir_json, tmpdir, neff_name="file.neff"):
        key = hashlib.sha256(bir_json).hexdigest()[:24]
        path = os.path.join(cache_dir, f"{key}.neff")
        if os.path.exists(path):
            dst = os.path.join(tmpdir, neff_name)
            with open(path, "rb") as f_in, open(dst, "wb") as f_out:
                f_out.write(f_in.read())
            return dst
        neff = real_compile(bir_json, tmpdir, neff_name)
        with open(neff, "rb") as f_in, open(path + ".tmp", "wb") as f_out:
            f_out.write(f_in.read())
        os.replace(path + ".tmp", path)
        return neff

    bass2jax.compile_bir_kernel = cached_compile
    bass2jax._neff_cache_installed = True


def _chunk_tasks(c):
    """QK/AV tasks for 512-query chunk c: list of (jb, off, width)."""
    tasks = []
    for jb in range(4 * c + 4):
        i_lo = max(jb * 128, c * 512)
        off = i_lo - c * 512
        tasks.append((jb, off, 512 - off))
    return tasks


def _slabs_for_chunk(c):
    """Pack chunk tasks into 1536-col (3 psum bank) slabs.

    Tasks are placed back-to-back; a task's start is rounded up to the next
    512 (psum bank) boundary if its output would otherwise cross one.
    Returns a list of slabs; each slab is a list of (jb, off, w, e0) with
    e0 = column offset inside the slab.
    """
    tasks = _chunk_tasks(c)
    slabs = []
    cur = []
    cap = 1536
    pos = 0
    for jb, off, w in tasks:
        start = pos
        if start // 512 != (start + w - 1) // 512:
            start = (start // 512 + 1) * 512
        if start + w > cap:
            slabs.append(cur)
            cur = []
            start = 0
        cur.append((jb, off, w, start))
        pos = start + w
    if cur:
        slabs.append(cur)
    return slabs


def _build():
    import concourse.bacc as bacc
    import concourse.mybir as mybir
    import concourse.tile as tile
    from concourse.masks import make_identity, make_upper_triangular

    f32 = mybir.dt.float32
    bf16 = mybir.dt.bfloat16
    f16 = mybir.dt.float16
    i16 = mybir.dt.int16
    Exp = mybir.ActivationFunctionType.Exp
    # scores arrive pre-scaled by SCHR_A; exact exp needs exp(s/8)
    EXP_SCALE = 0.125 / SCHR_A

    nc = bacc.Bacc()
    q_d = nc.dram_tensor("q", [HPC, 128, 2048], bf16, kind="ExternalInput")
    k1_d = nc.dram_tensor("k1", [HPC, 128, 1024], bf16, kind="ExternalInput")
    k2_d = nc.dram_tensor("k2", [HPC, 128, 1024], bf16, kind="ExternalInput")
    v_d = nc.dram_tensor("v", [HPC, S, D], f16, kind="ExternalInput")
    o_d = nc.dram_tensor("out", [HPC, 4, D + 1, 512], f32,
                         kind="ExternalOutput")

    with tile.TileContext(nc) as tc:
        with (
            tc.tile_pool(name="const", bufs=1) as const,
            tc.tile_pool(name="stage", bufs=3) as stage,
            tc.tile_pool(name="tdst", bufs=2) as tdst,
            tc.tile_pool(name="exps", bufs=12) as exps,
            tc.tile_pool(name="small", bufs=4) as small,
            tc.tile_pool(name="ps", bufs=1, space="PSUM") as ps,
        ):
            ident = const.tile([128, 128], f32)
            make_identity(nc, ident)
            # trimask[j, i] = 1.0 if j <= i else 0.0 (keep-mask, fp16)
            tri_f = const.tile([128, 128], f32)
            make_upper_triangular(nc, tri_f, val=1.0, diag=True)
            trimask16 = const.tile([128, 128], f16)
            nc.vector.tensor_copy(trimask16, tri_f)
            # per-slab-pattern 0/1 keep-masks for the DVE fast-exp:
            # scalar_tensor_tensor computes (s + SCHR_B) * mask in one
            # instruction, so masked elements become exact int16 0 =
            # fp16 +0.0 (the int16 convert WRAPS mod 2^16, so an additive
            # -inf style bias cannot work)
            bias_tiles = {}
            for c_ in (1, 2, 3):
                for si_, tasks_ in enumerate(_slabs_for_chunk(c_)):
                    if not any(jb * 128 == c_ * 512 + off
                               for jb, off, w, e0 in tasks_):
                        continue
                    end_ = tasks_[-1][3] + tasks_[-1][2]
                    bt = const.tile([128, end_], f16, name=f"bias_{c_}_{si_}")
                    nc.vector.memset(bt, 1.0)
                    for jb, off, w, e0 in tasks_:
                        if jb * 128 == c_ * 512 + off:
                            nc.vector.tensor_copy(
                                bt[:, e0 : e0 + 128], trimask16)
                    bias_tiles[(c_, si_)] = bt

            def emit_head_load(h):
                """DMA loads for head h, in first-use order (chunk 1 needs
                kd1 + q[512:1024] first). Each DMA is a full either-lane
                tile so both PE row groups unblock at the same instant."""
                qT2 = tdst.tile([128, 2048], bf16, name=f"qT{h}", tag="qT")
                kd1 = tdst.tile([128, 1024], bf16, name=f"k1{h}", tag="k1")
                kd2 = tdst.tile([128, 1024], bf16, name=f"k2{h}", tag="k2")
                nc.sync.dma_start(out=kd1, in_=k1_d[h])
                nc.sync.dma_start(out=qT2[:, 512:1024],
                                  in_=q_d[h][:, 512:1024])
                nc.sync.dma_start(out=kd2, in_=k2_d[h])
                nc.sync.dma_start(out=qT2[:, 1024:2048],
                                  in_=q_d[h][:, 1024:2048])
                nc.sync.dma_start(out=qT2[:, 0:512], in_=q_d[h][:, 0:512])
                vext = stage.tile([128, NB, 128], f16, name=f"vx{h}",
                                  tag="vx")
                nc.sync.dma_start(
                    out=vext[:, :, 0:D],
                    in_=v_d[h].rearrange("(n p) d -> p n d", p=128))
                nc.gpsimd.memset(vext[:, :, D], 1.0)
                nc.gpsimd.memset(vext[:, :, D + 1 : 128], 0.0)
                return (qT2, kd1, kd2, vext)

            def kT_slice(kd1, kd2, jb, lane):
                """lhsT AP for key block jb on row-group lane (0 or 64)."""
                src = kd1 if jb < 8 else kd2
                j = jb % 8
                return src[lane : lane + 64, j * 128 : (j + 1) * 128]

            def emit_epilogue(h, c, acc):
                """Drain a chunk's accumulator: one copy + DMA. The copy
                is high priority so the psum bank recycles ASAP, and goes
                to whichever exp engine is currently less loaded."""
                osb = small.tile([D + 1, 512], f32, tag="osb",
                                 name=f"osb_{h}_{c}")
                cost_sc = (512 + 172) / 1.2
                cost_dve = (512 + 120) / 0.96
                with tc.high_priority():
                    if state["sc_ns"] + cost_sc <= state["dve_ns"] + cost_dve:
                        nc.scalar.copy(osb, acc[0 : D + 1])
                        state["sc_ns"] += cost_sc
                    else:
                        nc.vector.tensor_copy(osb, acc[0 : D + 1])
                        state["dve_ns"] += cost_dve
                nc.sync.dma_start(out=o_d[h, c], in_=osb)

            def av_mm(acc, vslice, eslice, start, stop):
                def fn():
                    nc.tensor.matmul(
                        acc, vslice, eslice, start=start, stop=stop,
                        skip_group_check=True, tile_position=(0, 0))
                return fn

            def av_items(p):
                """Full-array AV closures for a finished slab (+epilogue)."""
                meta, eT, acc, ctx, _seq = p
                h, c, n_jb, vext = ctx
                avs = []
                epi = None
                for jb, off, w, e0 in meta:
                    avs.append(av_mm(
                        acc[:, off : off + w], vext[:, jb, :],
                        eT[:, e0 : e0 + w],
                        jb == 0, jb == n_jb - 1))
                    if jb == n_jb - 1:
                        epi = (h, c, acc)
                return avs, epi

            def qk_mm(slab, lhsT, rhs, lane):
                def fn():
                    nc.tensor.matmul(
                        slab, lhsT, rhs, start=True, stop=True,
                        skip_group_check=True, tile_position=(lane, 0))
                return fn

            # software pipeline state + engine load balance (ns estimates)
            state = {"pending": [], "sc_ns": 0.0, "dve_ns": 0.0,
                     "qk0": 0, "qk64": 0, "g0": [], "g64": [], "cur": 0,
                     "seq": 0, "exp_prev": None, "mask_i": 0}

            def pump():
                """Emit queued QK matmuls in strict 0/64 row-group
                alternation. Leftovers (when one lane's deque empties)
                stay queued and pair with the next slab's tasks."""
                g0, g64 = state["g0"], state["g64"]
                while g0 and g64:
                    if state["cur"] == 0:
                        g0.pop(0)[1]()
                        state["cur"] = 64
                    else:
                        g64.pop(0)[1]()
                        state["cur"] = 0

            def flush_upto(seq_limit):
                """Force-emit queued QKs from slabs <= seq_limit. Must run
                before a slab's exp: the exp reads the slab's psum banks,
                and a QK writer emitted AFTER the exp would race it (fatal
                TensorE-write/ScalarE-read collision)."""
                for q in (state["g0"], state["g64"]):
                    while q and q[0][0] <= seq_limit:
                        q.pop(0)[1]()

            def run_exp_prev():
                if state["exp_prev"] is not None:
                    s_seq, fn, epis = state["exp_prev"]
                    flush_upto(s_seq)
                    fn()
                    # epilogues here: flush_upto guarantees the chunk's
                    # final AV writer is emitted before the acc read
                    for epi in epis:
                        emit_epilogue(*epi)
                    state["exp_prev"] = None

            def flush():
                pump()
                for q in (state["g0"], state["g64"]):
                    while q:
                        q.pop(0)[1]()

            def emit_slab(h, c, slab_tasks, si, tiles, acc, n_jb):
                qT2, kd1, kd2, vext = tiles
                seq = state["seq"]
                state["seq"] += 1
                tag = "slabA" if seq % 2 == 0 else "slabB"
                slab = ps.tile([128, 1536], f32, tag=tag,
                               bufs=1, name=f"{tag}_{h}_{c}_{si}")
                eT = exps.tile([128, 1536], f16, tag="eT",
                               name=f"eT_{h}_{c}_{si}")
                # AV of slab s-LAG first (inputs long since ready, runs
                # full-array), then QK of this slab via the alternating
                # pairing deques.
                # batch two slabs' AVs on even seq (none on odd): halves
                # the number of AV<->QK phase transitions, each of which
                # costs ~100ns of PE drain/fill overlap loss
                epis = []
                if seq % 3 == 0:
                    while len(state["pending"]) > LAG - 2:
                        avs, epi = av_items(state["pending"].pop(0))
                        for fn in avs:
                            fn()
                        if epi is not None:
                            epis.append(epi)
                l0, l64 = state["g0"], state["g64"]
                # tasks sharing a psum bank must take the same row group:
                # two concurrent matmul streams into one single-port bank
                # SRAM is a hardware conflict
                groups = {}
                for t in slab_tasks:
                    groups.setdefault(t[3] // 512, []).append(t)
                for bank in sorted(groups):
                    lane = 0 if state["qk0"] <= state["qk64"] else 64
                    for jb, off, w, e0 in groups[bank]:
                        state["qk0" if lane == 0 else "qk64"] += w
                        q0 = c * 512 + off
                        mm = qk_mm(slab[:, e0 : e0 + w],
                                   kT_slice(kd1, kd2, jb, lane),
                                   qT2[lane : lane + 64, q0 : q0 + w], lane)
                        (l64 if lane else l0).append((seq, mm))
                pump()
                # exp of the PREVIOUS slab: deferred until now so that all
                # of its QK writers are guaranteed emitted first (Tile can
                # only order the exp's psum read after writers it has seen)
                run_exp_prev()
                # exp per contiguous written run (bank-alignment rounding
                # can leave gap columns that belong to the previous slab
                # tile -- reading those would race)
                runs = []
                for jb, off, w, e0 in slab_tasks:
                    if runs and runs[-1][1] == e0:
                        runs[-1][1] = e0 + w
                    else:
                        runs.append([e0, e0 + w])
                ncols = sum(b - a for a, b in runs)
                cost_sc = (ncols + 352 * len(runs)) / 1.2
                cost_dve = (ncols + 120 * len(runs)) / 0.96
                bias = bias_tiles.get((c, si))
                if c == 0:
                    use_sc = True
                    state["dve_ns"] += 127.0 * 2  # its masks, half to DVE
                elif bias is not None:
                    use_sc = False
                else:
                    use_sc = (state["sc_ns"] + cost_sc
                              <= state["dve_ns"] + cost_dve)
                if use_sc:
                    state["sc_ns"] += cost_sc
                else:
                    state["dve_ns"] += cost_dve

                def exp_fn(eT=eT, slab=slab, runs=runs, use_sc=use_sc,
                           slab_tasks=slab_tasks, c=c, bias=bias):
                    for a, b in runs:
                        if use_sc:
                            nc.scalar.activation(
                                eT[:, a:b], slab[:, a:b], Exp,
                                scale=EXP_SCALE)
                        elif bias is not None:
                            nc.vector.scalar_tensor_tensor(
                                eT[:, a:b].bitcast(i16),
                                slab[:, a:b], SCHR_B, bias[:, a:b],
                                mybir.AluOpType.add, mybir.AluOpType.mult)
                        else:
                            nc.vector.tensor_scalar(
                                eT[:, a:b].bitcast(i16), slab[:, a:b],
                                SCHR_B, None, mybir.AluOpType.add)
                    if bias is not None:
                        return
                    # mask diagonal 128-blocks (task start == diagonal),
                    # alternating between GpSimd and DVE so neither backs
                    # up the just-in-time AV consumers
                    for jb, off, w, e0 in slab_tasks:
                        if jb * 128 == c * 512 + off:
                            state["mask_i"] += 1
                            if state["mask_i"] % 2:
                                nc.gpsimd.tensor_mul(
                                    eT[:, e0 : e0 + 128],
                                    eT[:, e0 : e0 + 128],
                                    trimask16)
                            else:
                                nc.vector.tensor_mul(
                                    eT[:, e0 : e0 + 128],
                                    eT[:, e0 : e0 + 128],
                                    trimask16)

                # the epilogue rides along with the deferred exp: both may
                # only be emitted once every matmul of this slab is out
                state["exp_prev"] = (seq, exp_fn, epis)
                state["pending"].append((slab_tasks, eT, acc,
                                         (h, c, n_jb, vext), seq))

            # warm-up: matmuls spanning the head-0 load window keep the PE
            # activity monitor at full clock into the real stream; they
            # write one acc buffer (free until chunk 2's AVs). A dummy
            # ACTIVATE preloads the exp table set (~2.7us) off the
            # critical path.
            scr = const.tile([128, 512], bf16)
            nc.gpsimd.memset(scr, 0.0)
            tiles0 = emit_head_load(0)
            # two warmup tiles (one per row group): concurrent matmuls on
            # the two row groups must write different psum banks
            wu_a = ps.tile([128, 512], f32, tag="acc", bufs=2, name="wuA")
            wu_b = ps.tile([128, 512], f32, tag="acc", bufs=2, name="wuB")
            dummy_e = const.tile([128, 8], f16)
            nc.scalar.activation(dummy_e, ident[:, 0:8], Exp, scale=0.125)
            for i in range(12):
                lane_w = 64 * (i % 2)
                nc.tensor.matmul(
                    (wu_b if lane_w else wu_a)[0 : D + 1, 0:512],
                    scr[lane_w : lane_w + 64, 0:D + 1],
                    scr[lane_w : lane_w + 64, :],
                    start=True, stop=True,
                    skip_group_check=True, tile_position=(lane_w, 0))
            heads = {0: tiles0}
            for h in range(HPC):
                tiles = heads.pop(h)
                for ci, c in enumerate((1, 2, 3, 0)):
                    n_jb = 4 * c + 4
                    acc = ps.tile([128, 512], f32, tag="acc", bufs=2,
                                  name=f"acc_{h}_{c}")
                    for si, slab_tasks in enumerate(_slabs_for_chunk(c)):
                        emit_slab(h, c, slab_tasks, si, tiles, acc, n_jb)
                    # pipeline next head's loads after the first chunk
                    if h + 1 < HPC and ci == 0:
                        heads[h + 1] = emit_head_load(h + 1)
            # emit the final slab's exp (after force-draining its QKs),
            # then flush remaining pending slabs in two batches so the
            # last-but-one epilogue overlaps the final AVs
            flush_upto(state["seq"])
            run_exp_prev()

            def drain_batch(n, last):
                epis = []
                for _ in range(n):
                    avs, epi = av_items(state["pending"].pop(0))
                    for fn in avs:
                        fn()
                    if epi is not None:
                        epis.append(epi)
                if last:
                    flush()
                for epi in epis:
                    emit_epilogue(*epi)

            if len(state["pending"]) > 1:
                drain_batch(len(state["pending"]) - 1, last=False)
            drain_batch(len(state["pending"]), last=True)

    nc.finalize()
    return nc


def _get_nc():
    global _NC
    if _NC is None:
        _install_neff_cache()
        _NC = _build()
    return _NC


def _prep_in_maps(q, k, v):
    """Host-side layout prep -> per-core input maps."""
    import ml_dtypes

    bf = ml_dtypes.bfloat16
    q = np.asarray(q, dtype=np.float32).reshape(B * H, S, D).astype(bf)
    k = np.asarray(k, dtype=np.float32).reshape(B * H, S, D)
    # fold the Schraudolph input scale into k (bf16 precision is relative,
    # so this is free); the exact-exp path compensates via EXP_SCALE
    k = (k * SCHR_A).astype(bf)
    v = np.asarray(v, dtype=np.float32).reshape(B * H, S, D)
    v = v.astype(np.float16)
    # d-major layouts, duplicated on both partition halves
    qT = np.ascontiguousarray(q.transpose(0, 2, 1))  # [BH, 64, 2048]
    qdup = np.concatenate([qT, qT], axis=1)          # [BH, 128, 2048]
    kT = np.ascontiguousarray(
        k.reshape(B * H, 2, 1024, D).transpose(0, 1, 3, 2))  # [BH,2,64,1024]
    kd1 = np.concatenate([kT[:, 0], kT[:, 0]], axis=1)  # blocks 0-7 dup
    kd2 = np.concatenate([kT[:, 1], kT[:, 1]], axis=1)  # blocks 8-15 dup
    return [
        {
            "q": np.ascontiguousarray(qdup[c * HPC : (c + 1) * HPC]),
            "k1": np.ascontiguousarray(kd1[c * HPC : (c + 1) * HPC]),
            "k2": np.ascontiguousarray(kd2[c * HPC : (c + 1) * HPC]),
            "v": np.ascontiguousarray(v[c * HPC : (c + 1) * HPC]),
        }
        for c in range(N_CORES)
    ]


def _post(results):
    """Gather per-core raw outputs -> full [B, H, S, D] fp32."""
    raw = np.stack([results[c]["out"] for c in range(N_CORES)])
    raw = raw.reshape(B * H, 4, D + 1, 512)
    out = raw[:, :, :D, :] / raw[:, :, D : D + 1, :]
    out = out.transpose(0, 1, 3, 2)  # [BH, 4, 512, D]
    return out.reshape(B, H, S, D).astype(np.float32)


def kernel(q, k, v):
    from concourse.bass_utils import run_bass_kernel_spmd

    nc = _get_nc()
    in_maps = _prep_in_maps(q, k, v)
    res = run_bass_kernel_spmd(nc, in_maps, core_ids=list(range(N_CORES)))
    return _post(res.results)


# revision 39
# speedup vs baseline: 1.0128x; 1.0128x over previous
"""Chunked-causal attention (MemoryEfficientAttention) for Trainium2.

Full inputs q,k,v: [2, 16, 2048, 64] fp32. Causal attention per (batch, head).
Sharding: 32 (batch*head) slices split 4-per-core across 8 NeuronCores.

Per-core design (v4). Three engine budgets are balanced against each other:
PE ~45us of matmul columns + weight loads, and the ~8.9M-element exp over
causal scores split between ScalarE (exact ACTIVATE Exp) and DVE
(Schraudolph int16-bitcast fp16 approx) so neither engine exceeds ~50us.

  - Host prep (free off-chip): q,k cast to bf16, d-major, and DUPLICATED on
    both partition halves (qdup[128, 2048]; kd1 = key blocks 0-7 on both
    halves, kd2 = blocks 8-15). k is PRE-SCALED by SCHR_A so the DVE fast
    exp is a single-op tensor_scalar. v ships fp16. Per-tile DMAs are
    atomic, so both PE row groups become ready simultaneously (skewed
    readiness makes the Tile scheduler bunch one row group's matmuls,
    serializing the PE).
  - QK^T: K=64 matmuls on the two 64-row PE groups (tile_position (0,0) /
    (64,0)); the groups execute concurrently when consecutive instructions
    alternate row group, so QK tasks are assigned by a running
    column-balance and emitted strictly alternating (cross-slab leftover
    deques). Scores land in 512-col slots of two 3-bank psum slabs
    (double buffered).
  - exp: per slab, greedy-balanced between ScalarE ACTIVATE(Exp) -> fp16
    eT and DVE tensor_scalar add(SCHR_B) bitcast fp16 (Schraudolph fast
    exp, ~1.5% element error). Chunk-0 slabs (short softmax windows) are
    forced to the exact ScalarE path; measured whole-problem rel err ~6e-3
    vs the 2e-2 gate. Each slab's exp is DEFERRED one slab so every QK
    writer is emitted before the psum read (else: fatal TensorE-write /
    ScalarE-read bank collision). Diagonal 128x128 blocks masked by a
    triangular fp16 multiply on the otherwise-idle GpSimd engine.
  - AV: ONE full-array matmul per task (K=128 keys, M=65 with row 64 = the
    ones-column softmax denominator) into a double-buffered [65, 512]
    accumulator -- half the weight loads of a row-group-split AV, no
    combine add, and no write-after-read stall at chunk boundaries.
    Epilogue: one ScalarE copy psum->SBUF, DMA out fp32; divide +
    transpose on host.
  - Software pipeline: AV for slab s-LAG is emitted next to QK for slab s;
    heads prefetch one ahead; chunks run in order (1,2,3,0) so the tiny
    chunk 0 drains the pipeline. Warmup matmuls bridge the head-0 load
    window (PE clock gate needs ~3.4us of sustained activity); a dummy
    ACTIVATE preloads the exp table set.

Softmax computed without max-subtraction: scores/8 stay well inside
fp32/fp16 exp range for this problem family (|q.k|/8 <~ 6 -> exp <= ~400).
"""

import hashlib
import os

import numpy as np

B, H, S, D = 2, 16, 2048, 64
N_CORES = 8
HPC = (B * H) // N_CORES  # heads per core
NB = S // 128             # 16 key/query blocks per head
LAG = 4                   # AV trails QK by this many slabs

_NC = None

# fast-exp constants: int16(round(A*s + B)) bitcast fp16 ~= exp(s/8).
# A is folded into k on the host; the kernel sees pre-scaled scores.
SCHR_A = 0.125 * 1.4426950408889634 * 1024.0
SCHR_B = 15360.0 - 44.0


def _install_neff_cache():
    """Content-addressed NEFF cache so repeat runs skip the walrus compile."""
    import concourse.bass2jax as bass2jax

    real_compile = bass2jax.compile_bir_kernel
    if getattr(bass2jax, "_neff_cache_installed", False):
        return
    cache_dir = os.path.expanduser("~/.cache/bass_neff")
    os.makedirs(cache_dir, exist_ok=True)

    def cached_compile(bir_json, tmpdir, neff_name="file.neff"):
        key = hashlib.sha256(bir_json).hexdigest()[:24]
        path = os.path.join(cache_dir, f"{key}.neff")
        if os.path.exists(path):
            dst = os.path.join(tmpdir, neff_name)
            with open(path, "rb") as f_in, open(dst, "wb") as f_out:
                f_out.write(f_in.read())
            return dst
        neff = real_compile(bir_json, tmpdir, neff_name)
        with open(neff, "rb") as f_in, open(path + ".tmp", "wb") as f_out:
            f_out.write(f_in.read())
        os.replace(path + ".tmp", path)
        return neff

    bass2jax.compile_bir_kernel = cached_compile
    bass2jax._neff_cache_installed = True


def _chunk_tasks(c):
    """QK/AV tasks for 512-query chunk c: list of (jb, off, width)."""
    tasks = []
    for jb in range(4 * c + 4):
        i_lo = max(jb * 128, c * 512)
        off = i_lo - c * 512
        tasks.append((jb, off, 512 - off))
    return tasks


def _slabs_for_chunk(c):
    """Pack chunk tasks into 1536-col (3 psum bank) slabs.

    Tasks are placed back-to-back; a task's start is rounded up to the next
    512 (psum bank) boundary if its output would otherwise cross one.
    Returns a list of slabs; each slab is a list of (jb, off, w, e0) with
    e0 = column offset inside the slab.
    """
    tasks = _chunk_tasks(c)
    slabs = []
    cur = []
    cap = 1536
    pos = 0
    for jb, off, w in tasks:
        start = pos
        if start // 512 != (start + w - 1) // 512:
            start = (start // 512 + 1) * 512
        if start + w > cap:
            slabs.append(cur)
            cur = []
            start = 0
        cur.append((jb, off, w, start))
        pos = start + w
    if cur:
        slabs.append(cur)
    return slabs


def _build():
    import concourse.bacc as bacc
    import concourse.mybir as mybir
    import concourse.tile as tile
    from concourse.masks import make_identity, make_upper_triangular

    f32 = mybir.dt.float32
    bf16 = mybir.dt.bfloat16
    f16 = mybir.dt.float16
    i16 = mybir.dt.int16
    Exp = mybir.ActivationFunctionType.Exp
    # scores arrive pre-scaled by SCHR_A; exact exp needs exp(s/8)
    EXP_SCALE = 0.125 / SCHR_A

    nc = bacc.Bacc()
    q_d = nc.dram_tensor("q", [HPC, 128, 2048], bf16, kind="ExternalInput")
    k1_d = nc.dram_tensor("k1", [HPC, 128, 1024], bf16, kind="ExternalInput")
    k2_d = nc.dram_tensor("k2", [HPC, 128, 1024], bf16, kind="ExternalInput")
    v_d = nc.dram_tensor("v", [HPC, S, D], f16, kind="ExternalInput")
    o_d = nc.dram_tensor("out", [HPC, 4, D + 1, 512], f32,
                         kind="ExternalOutput")

    with tile.TileContext(nc) as tc:
        with (
            tc.tile_pool(name="const", bufs=1) as const,
            tc.tile_pool(name="stage", bufs=3) as stage,
            tc.tile_pool(name="tdst", bufs=2) as tdst,
            tc.tile_pool(name="exps", bufs=12) as exps,
            tc.tile_pool(name="small", bufs=4) as small,
            tc.tile_pool(name="ps", bufs=1, space="PSUM") as ps,
        ):
            ident = const.tile([128, 128], f32)
            make_identity(nc, ident)
            # trimask[j, i] = 1.0 if j <= i else 0.0 (keep-mask, fp16)
            tri_f = const.tile([128, 128], f32)
            make_upper_triangular(nc, tri_f, val=1.0, diag=True)
            trimask16 = const.tile([128, 128], f16)
            nc.vector.tensor_copy(trimask16, tri_f)
            # per-slab-pattern 0/1 keep-masks for the DVE fast-exp:
            # scalar_tensor_tensor computes (s + SCHR_B) * mask in one
            # instruction, so masked elements become exact int16 0 =
            # fp16 +0.0 (the int16 convert WRAPS mod 2^16, so an additive
            # -inf style bias cannot work)
            bias_tiles = {}
            for c_ in (1, 2, 3):
                for si_, tasks_ in enumerate(_slabs_for_chunk(c_)):
                    if not any(jb * 128 == c_ * 512 + off
                               for jb, off, w, e0 in tasks_):
                        continue
                    end_ = tasks_[-1][3] + tasks_[-1][2]
                    bt = const.tile([128, end_], f16, name=f"bias_{c_}_{si_}")
                    nc.vector.memset(bt, 1.0)
                    for jb, off, w, e0 in tasks_:
                        if jb * 128 == c_ * 512 + off:
                            nc.vector.tensor_copy(
                                bt[:, e0 : e0 + 128], trimask16)
                    bias_tiles[(c_, si_)] = bt

            def emit_head_load(h):
                """DMA loads for head h, in first-use order (chunk 1 needs
                kd1 + q[512:1024] first). Each DMA is a full either-lane
                tile so both PE row groups unblock at the same instant."""
                qT2 = tdst.tile([128, 2048], bf16, name=f"qT{h}", tag="qT")
                kd1 = tdst.tile([128, 1024], bf16, name=f"k1{h}", tag="k1")
                kd2 = tdst.tile([128, 1024], bf16, name=f"k2{h}", tag="k2")
                nc.sync.dma_start(out=kd1, in_=k1_d[h])
                nc.sync.dma_start(out=qT2[:, 512:1024],
                                  in_=q_d[h][:, 512:1024])
                nc.sync.dma_start(out=kd2, in_=k2_d[h])
                nc.sync.dma_start(out=qT2[:, 1024:2048],
                                  in_=q_d[h][:, 1024:2048])
                nc.sync.dma_start(out=qT2[:, 0:512], in_=q_d[h][:, 0:512])
                vext = stage.tile([128, NB, 128], f16, name=f"vx{h}",
                                  tag="vx")
                nc.sync.dma_start(
                    out=vext[:, :, 0:D],
                    in_=v_d[h].rearrange("(n p) d -> p n d", p=128))
                nc.gpsimd.memset(vext[:, :, D], 1.0)
                nc.gpsimd.memset(vext[:, :, D + 1 : 128], 0.0)
                return (qT2, kd1, kd2, vext)

            def kT_slice(kd1, kd2, jb, lane):
                """lhsT AP for key block jb on row-group lane (0 or 64)."""
                src = kd1 if jb < 8 else kd2
                j = jb % 8
                return src[lane : lane + 64, j * 128 : (j + 1) * 128]

            def emit_epilogue(h, c, acc):
                """Drain a chunk's accumulator: one copy + DMA. The copy
                is high priority so the psum bank recycles ASAP, and goes
                to whichever exp engine is currently less loaded."""
                osb = small.tile([D + 1, 512], f32, tag="osb",
                                 name=f"osb_{h}_{c}")
                cost_sc = (512 + 172) / 1.2
                cost_dve = (512 + 120) / 0.96
                with tc.high_priority():
                    if state["sc_ns"] + cost_sc <= state["dve_ns"] + cost_dve:
                        nc.scalar.copy(osb, acc[0 : D + 1])
                        state["sc_ns"] += cost_sc
                    else:
                        nc.vector.tensor_copy(osb, acc[0 : D + 1])
                        state["dve_ns"] += cost_dve
                nc.sync.dma_start(out=o_d[h, c], in_=osb)

            def av_mm(acc, vslice, eslice, start, stop):
                def fn():
                    nc.tensor.matmul(
                        acc, vslice, eslice, start=start, stop=stop,
                        skip_group_check=True, tile_position=(0, 0))
                return fn

            def av_items(p):
                """Full-array AV closures for a finished slab (+epilogue)."""
                meta, eT, acc, ctx, _seq = p
                h, c, n_jb, vext = ctx
                avs = []
                epi = None
                for jb, off, w, e0 in meta:
                    avs.append(av_mm(
                        acc[:, off : off + w], vext[:, jb, :],
                        eT[:, e0 : e0 + w],
                        jb == 0, jb == n_jb - 1))
                    if jb == n_jb - 1:
                        epi = (h, c, acc)
                return avs, epi

            def qk_mm(slab, lhsT, rhs, lane):
                def fn():
                    nc.tensor.matmul(
                        slab, lhsT, rhs, start=True, stop=True,
                        skip_group_check=True, tile_position=(lane, 0))
                return fn

            # software pipeline state + engine load balance (ns estimates)
            state = {"pending": [], "sc_ns": 0.0, "dve_ns": 0.0,
                     "qk0": 0, "qk64": 0, "g0": [], "g64": [], "cur": 0,
                     "seq": 0, "exp_prev": None, "mask_i": 0}

            def pump():
                """Emit queued QK matmuls in strict 0/64 row-group
                alternation. Leftovers (when one lane's deque empties)
                stay queued and pair with the next slab's tasks."""
                g0, g64 = state["g0"], state["g64"]
                while g0 and g64:
                    if state["cur"] == 0:
                        g0.pop(0)[1]()
                        state["cur"] = 64
                    else:
                        g64.pop(0)[1]()
                        state["cur"] = 0

            def flush_upto(seq_limit):
                """Force-emit queued QKs from slabs <= seq_limit. Must run
                before a slab's exp: the exp reads the slab's psum banks,
                and a QK writer emitted AFTER the exp would race it (fatal
                TensorE-write/ScalarE-read collision)."""
                for q in (state["g0"], state["g64"]):
                    while q and q[0][0] <= seq_limit:
                        q.pop(0)[1]()

            def run_exp_prev():
                if state["exp_prev"] is not None:
                    s_seq, fn, epis = state["exp_prev"]
                    flush_upto(s_seq)
                    fn()
                    # epilogues here: flush_upto guarantees the chunk's
                    # final AV writer is emitted before the acc read
                    for epi in epis:
                        emit_epilogue(*epi)
                    state["exp_prev"] = None

            def flush():
                pump()
                for q in (state["g0"], state["g64"]):
                    while q:
                        q.pop(0)[1]()

            def emit_slab(h, c, slab_tasks, si, tiles, acc, n_jb):
                qT2, kd1, kd2, vext = tiles
                seq = state["seq"]
                state["seq"] += 1
                tag = "slabA" if seq % 2 == 0 else "slabB"
                slab = ps.tile([128, 1536], f32, tag=tag,
                               bufs=1, name=f"{tag}_{h}_{c}_{si}")
                eT = exps.tile([128, 1536], f16, tag="eT",
                               name=f"eT_{h}_{c}_{si}")
                # AV of slab s-LAG first (inputs long since ready, runs
                # full-array), then QK of this slab via the alternating
                # pairing deques.
                # batch two slabs' AVs on even seq (none on odd): halves
                # the number of AV<->QK phase transitions, each of which
                # costs ~100ns of PE drain/fill overlap loss
                epis = []
                if seq % 2 == 0:
                    while len(state["pending"]) >= LAG:
                        avs, epi = av_items(state["pending"].pop(0))
                        for fn in avs:
                            fn()
                        if epi is not None:
                            epis.append(epi)
                l0, l64 = state["g0"], state["g64"]
                # tasks sharing a psum bank must take the same row group:
                # two concurrent matmul streams into one single-port bank
                # SRAM is a hardware conflict
                groups = {}
                for t in slab_tasks:
                    groups.setdefault(t[3] // 512, []).append(t)
                for bank in sorted(groups):
                    lane = 0 if state["qk0"] <= state["qk64"] else 64
                    for jb, off, w, e0 in groups[bank]:
                        state["qk0" if lane == 0 else "qk64"] += w
                        q0 = c * 512 + off
                        mm = qk_mm(slab[:, e0 : e0 + w],
                                   kT_slice(kd1, kd2, jb, lane),
                                   qT2[lane : lane + 64, q0 : q0 + w], lane)
                        (l64 if lane else l0).append((seq, mm))
                pump()
                # exp of the PREVIOUS slab: deferred until now so that all
                # of its QK writers are guaranteed emitted first (Tile can
                # only order the exp's psum read after writers it has seen)
                run_exp_prev()
                # exp per contiguous written run (bank-alignment rounding
                # can leave gap columns that belong to the previous slab
                # tile -- reading those would race)
                runs = []
                for jb, off, w, e0 in slab_tasks:
                    if runs and runs[-1][1] == e0:
                        runs[-1][1] = e0 + w
                    else:
                        runs.append([e0, e0 + w])
                ncols = sum(b - a for a, b in runs)
                cost_sc = (ncols + 352 * len(runs)) / 1.2
                cost_dve = (ncols + 120 * len(runs)) / 0.96
                bias = bias_tiles.get((c, si))
                if c == 0:
                    use_sc = True
                    state["dve_ns"] += 127.0 * 2  # its masks, half to DVE
                elif bias is not None:
                    use_sc = False
                else:
                    use_sc = (state["sc_ns"] + cost_sc
                              <= state["dve_ns"] + cost_dve)
                if use_sc:
                    state["sc_ns"] += cost_sc
                else:
                    state["dve_ns"] += cost_dve

                def exp_fn(eT=eT, slab=slab, runs=runs, use_sc=use_sc,
                           slab_tasks=slab_tasks, c=c, bias=bias):
                    for a, b in runs:
                        if use_sc:
                            nc.scalar.activation(
                                eT[:, a:b], slab[:, a:b], Exp,
                                scale=EXP_SCALE)
                        elif bias is not None:
                            nc.vector.scalar_tensor_tensor(
                                eT[:, a:b].bitcast(i16),
                                slab[:, a:b], SCHR_B, bias[:, a:b],
                                mybir.AluOpType.add, mybir.AluOpType.mult)
                        else:
                            nc.vector.tensor_scalar(
                                eT[:, a:b].bitcast(i16), slab[:, a:b],
                                SCHR_B, None, mybir.AluOpType.add)
                    if bias is not None:
                        return
                    # mask diagonal 128-blocks (task start == diagonal),
                    # alternating between GpSimd and DVE so neither backs
                    # up the just-in-time AV consumers
                    for jb, off, w, e0 in slab_tasks:
                        if jb * 128 == c * 512 + off:
                            state["mask_i"] += 1
                            if state["mask_i"] % 2:
                                nc.gpsimd.tensor_mul(
                                    eT[:, e0 : e0 + 128],
                                    eT[:, e0 : e0 + 128],
                                    trimask16)
                            else:
                                nc.vector.tensor_mul(
                                    eT[:, e0 : e0 + 128],
                                    eT[:, e0 : e0 + 128],
                                    trimask16)

                # the epilogue rides along with the deferred exp: both may
                # only be emitted once every matmul of this slab is out
                state["exp_prev"] = (seq, exp_fn, epis)
                state["pending"].append((slab_tasks, eT, acc,
                                         (h, c, n_jb, vext), seq))

            # warm-up: matmuls spanning the head-0 load window keep the PE
            # activity monitor at full clock into the real stream; they
            # write one acc buffer (free until chunk 2's AVs). A dummy
            # ACTIVATE preloads the exp table set (~2.7us) off the
            # critical path.
            scr = const.tile([128, 512], bf16)
            nc.gpsimd.memset(scr, 0.0)
            tiles0 = emit_head_load(0)
            # two warmup tiles (one per row group): concurrent matmuls on
            # the two row groups must write different psum banks
            wu_a = ps.tile([128, 512], f32, tag="acc", bufs=2, name="wuA")
            wu_b = ps.tile([128, 512], f32, tag="acc", bufs=2, name="wuB")
            dummy_e = const.tile([128, 8], f16)
            nc.scalar.activation(dummy_e, ident[:, 0:8], Exp, scale=0.125)
            for i in range(12):
                lane_w = 64 * (i % 2)
                nc.tensor.matmul(
                    (wu_b if lane_w else wu_a)[0 : D + 1, 0:512],
                    scr[lane_w : lane_w + 64, 0:D + 1],
                    scr[lane_w : lane_w + 64, :],
                    start=True, stop=True,
                    skip_group_check=True, tile_position=(lane_w, 0))
            heads = {0: tiles0}
            for h in range(HPC):
                tiles = heads.pop(h)
                for ci, c in enumerate((1, 2, 3, 0)):
                    n_jb = 4 * c + 4
                    acc = ps.tile([128, 512], f32, tag="acc", bufs=2,
                                  name=f"acc_{h}_{c}")
                    for si, slab_tasks in enumerate(_slabs_for_chunk(c)):
                        emit_slab(h, c, slab_tasks, si, tiles, acc, n_jb)
                    # pipeline next head's loads after the first chunk
                    if h + 1 < HPC and ci == 0:
                        heads[h + 1] = emit_head_load(h + 1)
            # emit the final slab's exp (after force-draining its QKs),
            # then flush remaining pending slabs in two batches so the
            # last-but-one epilogue overlaps the final AVs
            flush_upto(state["seq"])
            run_exp_prev()

            def drain_batch(n, last):
                epis = []
                for _ in range(n):
                    avs, epi = av_items(state["pending"].pop(0))
                    for fn in avs:
                        fn()
                    if epi is not None:
                        epis.append(epi)
                if last:
                    flush()
                for epi in epis:
                    emit_epilogue(*epi)

            if len(state["pending"]) > 1:
                drain_batch(len(state["pending"]) - 1, last=False)
            drain_batch(len(state["pending"]), last=True)

    nc.finalize()
    return nc


def _get_nc():
    global _NC
    if _NC is None:
        _install_neff_cache()
        _NC = _build()
    return _NC


def _prep_in_maps(q, k, v):
    """Host-side layout prep -> per-core input maps."""
    import ml_dtypes

    bf = ml_dtypes.bfloat16
    q = np.asarray(q, dtype=np.float32).reshape(B * H, S, D).astype(bf)
    k = np.asarray(k, dtype=np.float32).reshape(B * H, S, D)
    # fold the Schraudolph input scale into k (bf16 precision is relative,
    # so this is free); the exact-exp path compensates via EXP_SCALE
    k = (k * SCHR_A).astype(bf)
    v = np.asarray(v, dtype=np.float32).reshape(B * H, S, D)
    v = v.astype(np.float16)
    # d-major layouts, duplicated on both partition halves
    qT = np.ascontiguousarray(q.transpose(0, 2, 1))  # [BH, 64, 2048]
    qdup = np.concatenate([qT, qT], axis=1)          # [BH, 128, 2048]
    kT = np.ascontiguousarray(
        k.reshape(B * H, 2, 1024, D).transpose(0, 1, 3, 2))  # [BH,2,64,1024]
    kd1 = np.concatenate([kT[:, 0], kT[:, 0]], axis=1)  # blocks 0-7 dup
    kd2 = np.concatenate([kT[:, 1], kT[:, 1]], axis=1)  # blocks 8-15 dup
    return [
        {
            "q": np.ascontiguousarray(qdup[c * HPC : (c + 1) * HPC]),
            "k1": np.ascontiguousarray(kd1[c * HPC : (c + 1) * HPC]),
            "k2": np.ascontiguousarray(kd2[c * HPC : (c + 1) * HPC]),
            "v": np.ascontiguousarray(v[c * HPC : (c + 1) * HPC]),
        }
        for c in range(N_CORES)
    ]


def _post(results):
    """Gather per-core raw outputs -> full [B, H, S, D] fp32."""
    raw = np.stack([results[c]["out"] for c in range(N_CORES)])
    raw = raw.reshape(B * H, 4, D + 1, 512)
    out = raw[:, :, :D, :] / raw[:, :, D : D + 1, :]
    out = out.transpose(0, 1, 3, 2)  # [BH, 4, 512, D]
    return out.reshape(B, H, S, D).astype(np.float32)


def kernel(q, k, v):
    from concourse.bass_utils import run_bass_kernel_spmd

    nc = _get_nc()
    in_maps = _prep_in_maps(q, k, v)
    res = run_bass_kernel_spmd(nc, in_maps, core_ids=list(range(N_CORES)))
    return _post(res.results)


# revision 40
# speedup vs baseline: 1.0217x; 1.0088x over previous
"""Chunked-causal attention (MemoryEfficientAttention) for Trainium2.

Full inputs q,k,v: [2, 16, 2048, 64] fp32. Causal attention per (batch, head).
Sharding: 32 (batch*head) slices split 4-per-core across 8 NeuronCores.

Per-core design. Engine budgets balanced against each other: PE ~60us of
matmul columns + weight loads; the ~8.9M-element exp over causal scores
split between ScalarE (exact ACTIVATE Exp, ~45us) and DVE (Schraudolph
int16-bitcast fp16 approx, ~43us); diagonal masks fused into the DVE exp;
epilogue copies balanced across both. Measured ~87us vs the 101.5us
prior baseline; whole-problem rel err ~8e-3 vs the 2e-2 gate.

  - Host prep (free off-chip): q,k cast to bf16, d-major, and DUPLICATED on
    both partition halves (qdup[128, 2048]; kd1 = key blocks 0-7 on both
    halves, kd2 = blocks 8-15) so ANY QK task can run on EITHER PE row
    group. k is PRE-SCALED by SCHR_A so the DVE fast exp is a single-op
    tensor_scalar. v ships fp16. Per-tile DMAs are atomic so both PE row
    groups become ready simultaneously (skewed readiness makes the Tile
    scheduler bunch one row group's matmuls, serializing the PE -- the
    scheduler is a readiness+priority heap, NOT program order).
  - QK^T: K=64 matmuls on the two 64-row PE groups (tile_position (0,0) /
    (64,0)); the groups execute concurrently when consecutive instructions
    alternate row group, so QK tasks are assigned by a running
    column-balance (tasks sharing a psum bank take the same group -- two
    concurrent streams into one single-port bank SRAM is a fatal HW
    conflict) and emitted strictly alternating via cross-slab leftover
    deques. Scores land in 512-col slots of two 3-bank psum slabs.
  - exp: per slab, greedy-balanced between ScalarE ACTIVATE(Exp) -> fp16
    eT and DVE (s + SCHR_B) bitcast fp16 (Schraudolph, ~1.5% element
    error). Chunk-0 slabs (short softmax windows) forced to the exact
    ScalarE path. Slabs containing diagonal 128x128 blocks are forced to
    DVE where the causal mask is FUSED into the exp:
    scalar_tensor_tensor (s + SCHR_B) * keep01 makes masked elements
    exact int16 0 = fp16 +0.0 (the int16 convert WRAPS mod 2^16, so an
    additive -inf bias cannot work). Chunk-0's masks run as triangular
    multiplies alternating GpSimd/DVE. Each slab's exp is DEFERRED one
    slab so every QK writer is emitted before the psum read (else: fatal
    TensorE-write/ScalarE-read bank collision).
  - AV: ONE full-array matmul per task (K=128 keys; M=128 via vext padded
    to 128 value-columns, which qualifies the weight loads for Fast
    Weight Load; row 64 = the ones-column softmax denominator) into a
    double-buffered [128, 512] accumulator -- half the weight loads of a
    row-group-split AV, no combine add, no chunk-boundary stall. AV
    emission is batched two slabs at a time (fewer AV<->QK phase
    transitions, ~100ns each). Epilogue: one engine-balanced copy
    psum->SBUF at high priority, DMA out fp32; divide + transpose on host.
  - Software pipeline: AV for slab s-LAG rides next to QK for slab s;
    heads prefetch one ahead; chunks run in order (1,2,3,0) so the tiny
    chunk 0 drains the pipeline. Warmup matmuls bridge the head-0 load
    window (PE clock gate needs ~3.4us of sustained activity); a dummy
    ACTIVATE preloads the exp table set.

Softmax computed without max-subtraction: scores/8 stay well inside
fp32/fp16 exp range for this problem family (|q.k|/8 <~ 6 -> exp <= ~400).
"""

import hashlib
import os

import numpy as np

B, H, S, D = 2, 16, 2048, 64
N_CORES = 8
HPC = (B * H) // N_CORES  # heads per core
NB = S // 128             # 16 key/query blocks per head
LAG = 4                   # AV trails QK by this many slabs

_NC = None

# fast-exp constants: int16(round(A*s + B)) bitcast fp16 ~= exp(s/8).
# A is folded into k on the host; the kernel sees pre-scaled scores.
SCHR_A = 0.125 * 1.4426950408889634 * 1024.0
SCHR_B = 15360.0 - 44.0


def _install_neff_cache():
    """Content-addressed NEFF cache so repeat runs skip the walrus compile."""
    import concourse.bass2jax as bass2jax

    real_compile = bass2jax.compile_bir_kernel
    if getattr(bass2jax, "_neff_cache_installed", False):
        return
    cache_dir = os.path.expanduser("~/.cache/bass_neff")
    os.makedirs(cache_dir, exist_ok=True)

    def cached_compile(bir_json, tmpdir, neff_name="file.neff"):
        key = hashlib.sha256(bir_json).hexdigest()[:24]
        path = os.path.join(cache_dir, f"{key}.neff")
        if os.path.exists(path):
            dst = os.path.join(tmpdir, neff_name)
            with open(path, "rb") as f_in, open(dst, "wb") as f_out:
                f_out.write(f_in.read())
            return dst
        neff = real_compile(bir_json, tmpdir, neff_name)
        with open(neff, "rb") as f_in, open(path + ".tmp", "wb") as f_out:
            f_out.write(f_in.read())
        os.replace(path + ".tmp", path)
        return neff

    bass2jax.compile_bir_kernel = cached_compile
    bass2jax._neff_cache_installed = True


def _chunk_tasks(c):
    """QK/AV tasks for 512-query chunk c: list of (jb, off, width)."""
    tasks = []
    for jb in range(4 * c + 4):
        i_lo = max(jb * 128, c * 512)
        off = i_lo - c * 512
        tasks.append((jb, off, 512 - off))
    return tasks


def _slabs_for_chunk(c):
    """Pack chunk tasks into 1536-col (3 psum bank) slabs.

    Tasks are placed back-to-back; a task's start is rounded up to the next
    512 (psum bank) boundary if its output would otherwise cross one.
    Returns a list of slabs; each slab is a list of (jb, off, w, e0) with
    e0 = column offset inside the slab.
    """
    tasks = _chunk_tasks(c)
    slabs = []
    cur = []
    cap = 1536
    pos = 0
    for jb, off, w in tasks:
        start = pos
        if start // 512 != (start + w - 1) // 512:
            start = (start // 512 + 1) * 512
        if start + w > cap:
            slabs.append(cur)
            cur = []
            start = 0
        cur.append((jb, off, w, start))
        pos = start + w
    if cur:
        slabs.append(cur)
    return slabs


def _build():
    import concourse.bacc as bacc
    import concourse.mybir as mybir
    import concourse.tile as tile
    from concourse.masks import make_identity, make_upper_triangular

    f32 = mybir.dt.float32
    bf16 = mybir.dt.bfloat16
    f16 = mybir.dt.float16
    i16 = mybir.dt.int16
    Exp = mybir.ActivationFunctionType.Exp
    # scores arrive pre-scaled by SCHR_A; exact exp needs exp(s/8)
    EXP_SCALE = 0.125 / SCHR_A

    nc = bacc.Bacc()
    q_d = nc.dram_tensor("q", [HPC, 128, 2048], bf16, kind="ExternalInput")
    k1_d = nc.dram_tensor("k1", [HPC, 128, 1024], bf16, kind="ExternalInput")
    k2_d = nc.dram_tensor("k2", [HPC, 128, 1024], bf16, kind="ExternalInput")
    v_d = nc.dram_tensor("v", [HPC, S, D], f16, kind="ExternalInput")
    o_d = nc.dram_tensor("out", [HPC, 4, D + 1, 512], f32,
                         kind="ExternalOutput")

    with tile.TileContext(nc) as tc:
        with (
            tc.tile_pool(name="const", bufs=1) as const,
            tc.tile_pool(name="stage", bufs=3) as stage,
            tc.tile_pool(name="tdst", bufs=2) as tdst,
            tc.tile_pool(name="exps", bufs=12) as exps,
            tc.tile_pool(name="small", bufs=4) as small,
            tc.tile_pool(name="ps", bufs=1, space="PSUM") as ps,
        ):
            ident = const.tile([128, 128], f32)
            make_identity(nc, ident)
            # trimask[j, i] = 1.0 if j <= i else 0.0 (keep-mask, fp16)
            tri_f = const.tile([128, 128], f32)
            make_upper_triangular(nc, tri_f, val=1.0, diag=True)
            trimask16 = const.tile([128, 128], f16)
            nc.vector.tensor_copy(trimask16, tri_f)
            # per-slab-pattern 0/1 keep-masks for the DVE fast-exp:
            # scalar_tensor_tensor computes (s + SCHR_B) * mask in one
            # instruction, so masked elements become exact int16 0 =
            # fp16 +0.0 (the int16 convert WRAPS mod 2^16, so an additive
            # -inf style bias cannot work)
            bias_tiles = {}
            for c_ in (1, 2, 3):
                for si_, tasks_ in enumerate(_slabs_for_chunk(c_)):
                    if not any(jb * 128 == c_ * 512 + off
                               for jb, off, w, e0 in tasks_):
                        continue
                    end_ = tasks_[-1][3] + tasks_[-1][2]
                    bt = const.tile([128, end_], f16, name=f"bias_{c_}_{si_}")
                    nc.vector.memset(bt, 1.0)
                    for jb, off, w, e0 in tasks_:
                        if jb * 128 == c_ * 512 + off:
                            nc.vector.tensor_copy(
                                bt[:, e0 : e0 + 128], trimask16)
                    bias_tiles[(c_, si_)] = bt

            def emit_head_load(h):
                """DMA loads for head h, in first-use order (chunk 1 needs
                kd1 + q[512:1024] first). Each DMA is a full either-lane
                tile so both PE row groups unblock at the same instant."""
                qT2 = tdst.tile([128, 2048], bf16, name=f"qT{h}", tag="qT")
                kd1 = tdst.tile([128, 1024], bf16, name=f"k1{h}", tag="k1")
                kd2 = tdst.tile([128, 1024], bf16, name=f"k2{h}", tag="k2")
                nc.sync.dma_start(out=kd1, in_=k1_d[h])
                nc.sync.dma_start(out=qT2[:, 512:1024],
                                  in_=q_d[h][:, 512:1024])
                nc.sync.dma_start(out=kd2, in_=k2_d[h])
                nc.sync.dma_start(out=qT2[:, 1024:2048],
                                  in_=q_d[h][:, 1024:2048])
                nc.sync.dma_start(out=qT2[:, 0:512], in_=q_d[h][:, 0:512])
                vext = stage.tile([128, NB, 128], f16, name=f"vx{h}",
                                  tag="vx")
                nc.sync.dma_start(
                    out=vext[:, :, 0:D],
                    in_=v_d[h].rearrange("(n p) d -> p n d", p=128))
                nc.gpsimd.memset(vext[:, :, D], 1.0)
                nc.gpsimd.memset(vext[:, :, D + 1 : 128], 0.0)
                return (qT2, kd1, kd2, vext)

            def kT_slice(kd1, kd2, jb, lane):
                """lhsT AP for key block jb on row-group lane (0 or 64)."""
                src = kd1 if jb < 8 else kd2
                j = jb % 8
                return src[lane : lane + 64, j * 128 : (j + 1) * 128]

            def emit_epilogue(h, c, acc):
                """Drain a chunk's accumulator: one copy + DMA. The copy
                is high priority so the psum bank recycles ASAP, and goes
                to whichever exp engine is currently less loaded."""
                osb = small.tile([D + 1, 512], f32, tag="osb",
                                 name=f"osb_{h}_{c}")
                cost_sc = (512 + 172) / 1.2
                cost_dve = (512 + 120) / 0.96
                with tc.high_priority():
                    if state["sc_ns"] + cost_sc <= state["dve_ns"] + cost_dve:
                        nc.scalar.copy(osb, acc[0 : D + 1])
                        state["sc_ns"] += cost_sc
                    else:
                        nc.vector.tensor_copy(osb, acc[0 : D + 1])
                        state["dve_ns"] += cost_dve
                nc.sync.dma_start(out=o_d[h, c], in_=osb)

            def av_mm(acc, vslice, eslice, start, stop):
                def fn():
                    nc.tensor.matmul(
                        acc, vslice, eslice, start=start, stop=stop,
                        skip_group_check=True, tile_position=(0, 0))
                return fn

            def av_items(p):
                """Full-array AV closures for a finished slab (+epilogue)."""
                meta, eT, acc, ctx, _seq = p
                h, c, n_jb, vext = ctx
                avs = []
                epi = None
                for jb, off, w, e0 in meta:
                    avs.append(av_mm(
                        acc[:, off : off + w], vext[:, jb, :],
                        eT[:, e0 : e0 + w],
                        jb == 0, jb == n_jb - 1))
                    if jb == n_jb - 1:
                        epi = (h, c, acc)
                return avs, epi

            def qk_mm(slab, lhsT, rhs, lane):
                def fn():
                    nc.tensor.matmul(
                        slab, lhsT, rhs, start=True, stop=True,
                        skip_group_check=True, tile_position=(lane, 0))
                return fn

            # software pipeline state + engine load balance (ns estimates)
            state = {"pending": [], "sc_ns": 0.0, "dve_ns": 0.0,
                     "qk0": 0, "qk64": 0, "g0": [], "g64": [], "cur": 0,
                     "seq": 0, "exp_prev": None, "mask_i": 0}

            def pump():
                """Emit queued QK matmuls in strict 0/64 row-group
                alternation. Leftovers (when one lane's deque empties)
                stay queued and pair with the next slab's tasks."""
                g0, g64 = state["g0"], state["g64"]
                while g0 and g64:
                    if state["cur"] == 0:
                        g0.pop(0)[1]()
                        state["cur"] = 64
                    else:
                        g64.pop(0)[1]()
                        state["cur"] = 0

            def flush_upto(seq_limit):
                """Force-emit queued QKs from slabs <= seq_limit. Must run
                before a slab's exp: the exp reads the slab's psum banks,
                and a QK writer emitted AFTER the exp would race it (fatal
                TensorE-write/ScalarE-read collision)."""
                for q in (state["g0"], state["g64"]):
                    while q and q[0][0] <= seq_limit:
                        q.pop(0)[1]()

            def run_exp_prev():
                if state["exp_prev"] is not None:
                    s_seq, fn, epis = state["exp_prev"]
                    flush_upto(s_seq)
                    fn()
                    # epilogues here: flush_upto guarantees the chunk's
                    # final AV writer is emitted before the acc read
                    for epi in epis:
                        emit_epilogue(*epi)
                    state["exp_prev"] = None

            def flush():
                pump()
                for q in (state["g0"], state["g64"]):
                    while q:
                        q.pop(0)[1]()

            def emit_slab(h, c, slab_tasks, si, tiles, acc, n_jb):
                qT2, kd1, kd2, vext = tiles
                seq = state["seq"]
                state["seq"] += 1
                tag = "slabA" if seq % 2 == 0 else "slabB"
                slab = ps.tile([128, 1536], f32, tag=tag,
                               bufs=1, name=f"{tag}_{h}_{c}_{si}")
                eT = exps.tile([128, 1536], f16, tag="eT",
                               name=f"eT_{h}_{c}_{si}")
                # AV of slab s-LAG first (inputs long since ready, runs
                # full-array), then QK of this slab via the alternating
                # pairing deques.
                # batch two slabs' AVs on even seq (none on odd): halves
                # the number of AV<->QK phase transitions, each of which
                # costs ~100ns of PE drain/fill overlap loss
                epis = []
                if seq % 2 == 0:
                    while len(state["pending"]) >= LAG:
                        avs, epi = av_items(state["pending"].pop(0))
                        for fn in avs:
                            fn()
                        if epi is not None:
                            epis.append(epi)
                l0, l64 = state["g0"], state["g64"]
                # tasks sharing a psum bank must take the same row group:
                # two concurrent matmul streams into one single-port bank
                # SRAM is a hardware conflict
                groups = {}
                for t in slab_tasks:
                    groups.setdefault(t[3] // 512, []).append(t)
                for bank in sorted(groups):
                    lane = 0 if state["qk0"] <= state["qk64"] else 64
                    for jb, off, w, e0 in groups[bank]:
                        state["qk0" if lane == 0 else "qk64"] += w
                        q0 = c * 512 + off
                        mm = qk_mm(slab[:, e0 : e0 + w],
                                   kT_slice(kd1, kd2, jb, lane),
                                   qT2[lane : lane + 64, q0 : q0 + w], lane)
                        (l64 if lane else l0).append((seq, mm))
                pump()
                # exp of the PREVIOUS slab: deferred until now so that all
                # of its QK writers are guaranteed emitted first (Tile can
                # only order the exp's psum read after writers it has seen)
                run_exp_prev()
                # exp per contiguous written run (bank-alignment rounding
                # can leave gap columns that belong to the previous slab
                # tile -- reading those would race)
                runs = []
                for jb, off, w, e0 in slab_tasks:
                    if runs and runs[-1][1] == e0:
                        runs[-1][1] = e0 + w
                    else:
                        runs.append([e0, e0 + w])
                ncols = sum(b - a for a, b in runs)
                cost_sc = (ncols + 352 * len(runs)) / 1.2
                cost_dve = (ncols + 120 * len(runs)) / 0.96
                bias = bias_tiles.get((c, si))
                if c == 0:
                    use_sc = True
                    state["dve_ns"] += 127.0 * 2  # its masks, half to DVE
                elif bias is not None:
                    use_sc = False
                else:
                    use_sc = (state["sc_ns"] + cost_sc
                              <= state["dve_ns"] + cost_dve)
                if use_sc:
                    state["sc_ns"] += cost_sc
                else:
                    state["dve_ns"] += cost_dve

                def exp_fn(eT=eT, slab=slab, runs=runs, use_sc=use_sc,
                           slab_tasks=slab_tasks, c=c, bias=bias):
                    for a, b in runs:
                        if use_sc:
                            nc.scalar.activation(
                                eT[:, a:b], slab[:, a:b], Exp,
                                scale=EXP_SCALE)
                        elif bias is not None:
                            nc.vector.scalar_tensor_tensor(
                                eT[:, a:b].bitcast(i16),
                                slab[:, a:b], SCHR_B, bias[:, a:b],
                                mybir.AluOpType.add, mybir.AluOpType.mult)
                        else:
                            nc.vector.tensor_scalar(
                                eT[:, a:b].bitcast(i16), slab[:, a:b],
                                SCHR_B, None, mybir.AluOpType.add)
                    if bias is not None:
                        return
                    # mask diagonal 128-blocks (task start == diagonal),
                    # alternating between GpSimd and DVE so neither backs
                    # up the just-in-time AV consumers
                    for jb, off, w, e0 in slab_tasks:
                        if jb * 128 == c * 512 + off:
                            state["mask_i"] += 1
                            if state["mask_i"] % 2:
                                nc.gpsimd.tensor_mul(
                                    eT[:, e0 : e0 + 128],
                                    eT[:, e0 : e0 + 128],
                                    trimask16)
                            else:
                                nc.vector.tensor_mul(
                                    eT[:, e0 : e0 + 128],
                                    eT[:, e0 : e0 + 128],
                                    trimask16)

                # the epilogue rides along with the deferred exp: both may
                # only be emitted once every matmul of this slab is out
                state["exp_prev"] = (seq, exp_fn, epis)
                state["pending"].append((slab_tasks, eT, acc,
                                         (h, c, n_jb, vext), seq))

            # warm-up: matmuls spanning the head-0 load window keep the PE
            # activity monitor at full clock into the real stream; they
            # write one acc buffer (free until chunk 2's AVs). A dummy
            # ACTIVATE preloads the exp table set (~2.7us) off the
            # critical path.
            scr = const.tile([128, 512], bf16)
            nc.gpsimd.memset(scr, 0.0)
            tiles0 = emit_head_load(0)
            # two warmup tiles (one per row group): concurrent matmuls on
            # the two row groups must write different psum banks
            wu_a = ps.tile([128, 512], f32, tag="acc", bufs=2, name="wuA")
            wu_b = ps.tile([128, 512], f32, tag="acc", bufs=2, name="wuB")
            dummy_e = const.tile([128, 8], f16)
            nc.scalar.activation(dummy_e, ident[:, 0:8], Exp, scale=0.125)
            for i in range(12):
                lane_w = 64 * (i % 2)
                nc.tensor.matmul(
                    (wu_b if lane_w else wu_a)[0 : D + 1, 0:512],
                    scr[lane_w : lane_w + 64, 0:D + 1],
                    scr[lane_w : lane_w + 64, :],
                    start=True, stop=True,
                    skip_group_check=True, tile_position=(lane_w, 0))
            heads = {0: tiles0}
            for h in range(HPC):
                tiles = heads.pop(h)
                for ci, c in enumerate((1, 2, 3, 0)):
                    n_jb = 4 * c + 4
                    acc = ps.tile([128, 512], f32, tag="acc", bufs=2,
                                  name=f"acc_{h}_{c}")
                    for si, slab_tasks in enumerate(_slabs_for_chunk(c)):
                        emit_slab(h, c, slab_tasks, si, tiles, acc, n_jb)
                    # pipeline next head's loads after the first chunk
                    if h + 1 < HPC and ci == 0:
                        heads[h + 1] = emit_head_load(h + 1)
            # emit the final slab's exp (after force-draining its QKs),
            # then flush remaining pending slabs in two batches so the
            # last-but-one epilogue overlaps the final AVs
            flush_upto(state["seq"])
            run_exp_prev()

            def drain_batch(n, last):
                epis = []
                for _ in range(n):
                    avs, epi = av_items(state["pending"].pop(0))
                    for fn in avs:
                        fn()
                    if epi is not None:
                        epis.append(epi)
                if last:
                    flush()
                for epi in epis:
                    emit_epilogue(*epi)

            if len(state["pending"]) > 1:
                drain_batch(len(state["pending"]) - 1, last=False)
            drain_batch(len(state["pending"]), last=True)

    nc.finalize()
    return nc


def _get_nc():
    global _NC
    if _NC is None:
        _install_neff_cache()
        _NC = _build()
    return _NC


def _prep_in_maps(q, k, v):
    """Host-side layout prep -> per-core input maps."""
    import ml_dtypes

    bf = ml_dtypes.bfloat16
    q = np.asarray(q, dtype=np.float32).reshape(B * H, S, D).astype(bf)
    k = np.asarray(k, dtype=np.float32).reshape(B * H, S, D)
    # fold the Schraudolph input scale into k (bf16 precision is relative,
    # so this is free); the exact-exp path compensates via EXP_SCALE
    k = (k * SCHR_A).astype(bf)
    v = np.asarray(v, dtype=np.float32).reshape(B * H, S, D)
    v = v.astype(np.float16)
    # d-major layouts, duplicated on both partition halves
    qT = np.ascontiguousarray(q.transpose(0, 2, 1))  # [BH, 64, 2048]
    qdup = np.concatenate([qT, qT], axis=1)          # [BH, 128, 2048]
    kT = np.ascontiguousarray(
        k.reshape(B * H, 2, 1024, D).transpose(0, 1, 3, 2))  # [BH,2,64,1024]
    kd1 = np.concatenate([kT[:, 0], kT[:, 0]], axis=1)  # blocks 0-7 dup
    kd2 = np.concatenate([kT[:, 1], kT[:, 1]], axis=1)  # blocks 8-15 dup
    return [
        {
            "q": np.ascontiguousarray(qdup[c * HPC : (c + 1) * HPC]),
            "k1": np.ascontiguousarray(kd1[c * HPC : (c + 1) * HPC]),
            "k2": np.ascontiguousarray(kd2[c * HPC : (c + 1) * HPC]),
            "v": np.ascontiguousarray(v[c * HPC : (c + 1) * HPC]),
        }
        for c in range(N_CORES)
    ]


def _post(results):
    """Gather per-core raw outputs -> full [B, H, S, D] fp32."""
    raw = np.stack([results[c]["out"] for c in range(N_CORES)])
    raw = raw.reshape(B * H, 4, D + 1, 512)
    out = raw[:, :, :D, :] / raw[:, :, D : D + 1, :]
    out = out.transpose(0, 1, 3, 2)  # [BH, 4, 512, D]
    return out.reshape(B, H, S, D).astype(np.float32)


def kernel(q, k, v):
    from concourse.bass_utils import run_bass_kernel_spmd

    nc = _get_nc()
    in_maps = _prep_in_maps(q, k, v)
    res = run_bass_kernel_spmd(nc, in_maps, core_ids=list(range(N_CORES)))
    return _post(res.results)
